# revision 12
# baseline (speedup 1.0000x reference)
"""Tensor-parallel GQA multi-head-attention kernel for 8 trn2 NeuronCores.

Problem: B=2, T=2048, D=2048, H=16 q-heads, KV=4 kv-heads, HD=128,
causal attention with interleaved RoPE, y = attn_out @ Wo.

Sharding (tensor-parallel over heads, per the hint):
  core c = b*4 + g   (b = batch index, g = kv-head / q-head-group index)
  Each core computes q-heads 4g..4g+3 and kv-head g for batch b, plus the
  partial output  y_partial = attn_heads @ Wo[rows of those heads]  (row-
  parallel Wo).  The host sums the 4 partials per batch (the unshard of the
  row-parallel all-reduce) and stacks the 2 batches.

On-chip design (per core, everything bf16 except PSUM/softmax math):
  - host pre-transposes x -> xT [D,T] and permutes Wq/Wk columns per head to
    [even dims | odd dims] so RoPE pairs live in partition halves.
  - projections: q^T[h] = Wq_h^T @ xT  (lhsT=Wq chunk), k^T likewise,
    v natural via lhsT = xT block.
  - RoPE: rot = q*cos_dup + swap(q)*[-sin|sin]; the half-swap is an
    SBUF->SBUF DMA, the rest DVE.
  - attention per (head, 512-wide q chunk): for each 128-row k tile
    S^T = k^T_tile.T(dot) q^T chunk -> PSUM [128,512]; diagonal blocks get a
    -30000 mask add (DVE); ACT computes P = exp(scale*S^T) -> SBUF bf16;
    PV accumulates out^T[HD,512] with lhsT = v tile; an all-ones [128,128]
    lhsT matmul accumulates the softmax denominators broadcast across all
    128 partitions; normalization = reciprocal + one DVE multiply.
    Fully-masked (future) blocks are skipped -> ~40% less attention work.
  - Wo: y tile [128,512] = sum_h attnT_h chunk.T @ Wo_h chunk, DVE copy to
    SBUF, DMA to DRAM.
"""

import math

import numpy as np

B, T, D = 2, 2048, 2048
H, KV, HD = 16, 4, 128
ROPE_BASE = 10000.0
N_CORES = 8
HEADS_PER_CORE = H // KV // (N_CORES // (B * KV)) if False else 4  # 4
DQ = HEADS_PER_CORE * HD  # 512 q-dims per core
SCALE = 1.0 / math.sqrt(HD)
MASK_VAL = -30000.0

_CACHE = {}


def _build_nc(t_len=T):
    """Build the single-core SPMD Bass/Tile program (cached)."""
    import concourse.bass as bass
    import concourse.mybir as mybir
    import concourse.tile as tile
    from concourse import bacc

    f32 = mybir.dt.float32
    bf16 = mybir.dt.bfloat16
    ts = bass.ts

    NT = t_len // 128        # number of 128-row T tiles
    NK = D // 128            # contraction chunks for projections
    NCQ = t_len // 512       # number of 512-wide q chunks

    nc = bacc.Bacc("TRN2", target_bir_lowering=False, debug=False,
                   num_devices=N_CORES)

    xT_d = nc.dram_tensor("xT", [D, t_len], bf16, kind="ExternalInput").ap()
    wq_d = nc.dram_tensor("wq", [D, DQ], bf16, kind="ExternalInput").ap()
    wk_d = nc.dram_tensor("wk", [D, HD], bf16, kind="ExternalInput").ap()
    wv_d = nc.dram_tensor("wv", [D, HD], bf16, kind="ExternalInput").ap()
    wo_d = nc.dram_tensor("wo", [DQ, D], bf16, kind="ExternalInput").ap()
    cos_d = nc.dram_tensor("cosd", [128, t_len], bf16, kind="ExternalInput").ap()
    ssig_d = nc.dram_tensor("ssig", [128, t_len], bf16, kind="ExternalInput").ap()
    mask_d = nc.dram_tensor("mask", [128, 128], bf16, kind="ExternalInput").ap()
    y_d = nc.dram_tensor("y", [t_len, D], f32, kind="ExternalOutput").ap()

    Exp = mybir.ActivationFunctionType.Exp

    with tile.TileContext(nc) as tc:
        with (
            tc.tile_pool(name="const", bufs=1) as const,
            tc.tile_pool(name="qkv", bufs=1) as qkv,
            tc.tile_pool(name="attn", bufs=2) as attn_pool,
            tc.tile_pool(name="p", bufs=4) as p_pool,
            tc.tile_pool(name="rope", bufs=2) as rope_pool,
            tc.tile_pool(name="recip", bufs=2) as recip_pool,
            tc.tile_pool(name="y", bufs=3) as y_pool,
            tc.tile_pool(name="psum", bufs=1, space="PSUM") as psum,
        ):
            # ---- constant / input loads (per-k-chunk tiles so compute can
            # start as soon as the first chunks land) ----
            xT = [const.tile([128, t_len], bf16, tag=f"xT{k}", name=f"xT{k}") for k in range(NK)]
            wq = [const.tile([128, DQ], bf16, tag=f"wq{k}", name=f"wq{k}") for k in range(NK)]
            wk = [const.tile([128, HD], bf16, tag=f"wk{k}", name=f"wk{k}") for k in range(NK)]
            wv = [const.tile([128, HD], bf16, tag=f"wv{k}", name=f"wv{k}") for k in range(NK)]
            # loads ordered by first-use: wk, xT chunk0, rope tables c0,
            # wq, wv, wo, then the remaining xT chunks
            cos_sb = const.tile([128, t_len], bf16, tag="cos")
            ssig_sb = const.tile([128, t_len], bf16, tag="ssig")
            mask_sb = const.tile([128, 128], bf16, tag="mask")
            for k in range(NK):
                nc.sync.dma_start(wk[k][:], wk_d[ts(k, 128), :])
            for k in range(NK):
                nc.sync.dma_start(xT[k][:, 0:512], xT_d[ts(k, 128), 0:512])
            nc.sync.dma_start(cos_sb[:, 0:512], cos_d[:, 0:512])
            nc.sync.dma_start(ssig_sb[:, 0:512], ssig_d[:, 0:512])
            nc.sync.dma_start(mask_sb[:], mask_d[:])
            for k in range(NK):
                nc.sync.dma_start(wv[k][:], wv_d[ts(k, 128), :])
            for k in range(NK):
                nc.sync.dma_start(wq[k][:], wq_d[ts(k, 128), :])
            wo = [const.tile([128, D], bf16, tag=f"wo{h}", name=f"wo{h}") for h in range(4)]
            for h in range(4):
                nc.sync.dma_start(wo[h][:], wo_d[ts(h, 128), :])
            for c in range(1, NCQ):
                for k in range(NK):
                    nc.sync.dma_start(xT[k][:, ts(c, 512)], xT_d[ts(k, 128), ts(c, 512)])
                nc.sync.dma_start(cos_sb[:, ts(c, 512)], cos_d[:, ts(c, 512)])
                nc.sync.dma_start(ssig_sb[:, ts(c, 512)], ssig_d[:, ts(c, 512)])
            ones_sb = const.tile([128, 128], bf16, tag="ones")
            nc.vector.memset(ones_sb[:], 1.0)

            # persistent activations
            qT = qkv.tile([128, HEADS_PER_CORE, t_len], bf16, tag="qT")
            kT = qkv.tile([128, t_len], bf16, tag="kT")
            v_sb = qkv.tile([128, NT, HD], bf16, tag="v")

            def rope_to(dst_ap, psum_tile, c):
                """Apply RoPE to a [128, 512] psum tile (rows = [even|odd]
                dims of one head, cols = T positions of chunk c); write bf16
                to dst_ap."""
                cs = slice(c * 512, (c + 1) * 512)
                qf = rope_pool.tile([128, 512], f32, tag="qf")
                nc.vector.tensor_copy(qf[:], psum_tile[:])
                qs = rope_pool.tile([128, 512], f32, tag="qs")
                nc.gpsimd.dma_start(qs[0:64, :], qf[64:128, :])
                nc.gpsimd.dma_start(qs[64:128, :], qf[0:64, :])
                nc.vector.tensor_mul(qf[:], qf[:], cos_sb[:, cs])
                nc.vector.tensor_mul(qs[:], qs[:], ssig_sb[:, cs])
                nc.vector.tensor_add(dst_ap, qf[:], qs[:])

            def proj_chunk(c):
                """Projections for T positions [c*512, (c+1)*512)."""
                cs = slice(c * 512, (c + 1) * 512)
                # k^T chunk
                kp = psum.tile([128, 512], f32, tag="proj", bufs=2)
                for k in range(NK):
                    nc.tensor.matmul(kp[:], wk[k][:], xT[k][:, cs],
                                     start=(k == 0), stop=(k == NK - 1))
                rope_to(kT[:, cs], kp, c)
                # v tiles (natural layout), 4 per chunk
                for tt in range(4 * c, 4 * c + 4):
                    vp_full = psum.tile([128, 512], f32, tag="proj", bufs=2, name="vp")
                    vp = vp_full[:, :128]
                    for k in range(NK):
                        nc.tensor.matmul(vp[:], xT[k][:, ts(tt, 128)],
                                         wv[k][:],
                                         start=(k == 0), stop=(k == NK - 1))
                    nc.vector.tensor_copy(v_sb[:, tt, :], vp[:])
                # q^T chunks, one per head
                for h in range(HEADS_PER_CORE):
                    qp = psum.tile([128, 512], f32, tag="proj", bufs=2)
                    for k in range(NK):
                        nc.tensor.matmul(qp[:], wq[k][:, ts(h, 128)],
                                         xT[k][:, cs],
                                         start=(k == 0), stop=(k == NK - 1))
                    rope_to(qT[:, h, cs], qp, c)

            def attn_chunk(c):
                """Attention for q chunk c (all 4 heads) -> attnT tile."""
                attn_t = attn_pool.tile([128, HEADS_PER_CORE, 512], bf16,
                                        tag="attnT")
                nj = 4 * c + 4
                for h in range(HEADS_PER_CORE):
                    out_ps = psum.tile([128, 512], f32, tag="out", bufs=2)
                    sums_ps = psum.tile([128, 512], f32, tag="sums", bufs=1)
                    for j in range(nj):
                        # columns < o*128 of this [tk-tile, q-chunk] block
                        # are fully masked (tk > tq): skip them everywhere
                        o = j - 4 * c
                        lo = max(o, 0) * 128
                        qs0 = c * 512 + lo
                        s_ps = psum.tile([128, 512], f32, tag="s", bufs=3)
                        nc.tensor.matmul(s_ps[:, lo:], kT[:, ts(j, 128)],
                                         qT[:, h, qs0:(c + 1) * 512],
                                         start=True, stop=True)
                        if o >= 0:
                            nc.vector.tensor_add(s_ps[:, lo:lo + 128],
                                                 s_ps[:, lo:lo + 128],
                                                 mask_sb[:])
                        p = p_pool.tile([128, 512], bf16, tag="p")
                        nc.scalar.activation(p[:, lo:], s_ps[:, lo:], Exp,
                                             bias=0.0, scale=SCALE)
                        nc.tensor.matmul(out_ps[:, lo:], v_sb[:, j, :],
                                         p[:, lo:],
                                         start=(j == 0), stop=(j == nj - 1))
                        nc.tensor.matmul(sums_ps[:, lo:], ones_sb[:],
                                         p[:, lo:],
                                         start=(j == 0), stop=(j == nj - 1))
                    rc = recip_pool.tile([128, 512], f32, tag="rc")
                    nc.vector.reciprocal_approx_fast(out=rc[:], in_=sums_ps[:])
                    nc.vector.tensor_mul(attn_t[:, h, :], out_ps[:], rc[:])
                return attn_t

            def wo_chunk(c, attn_t):
                """Output projection for q chunk c."""
                for tq in range(4):
                    row0 = (4 * c + tq) * 128
                    for nn in range(4):
                        yp = psum.tile([128, 512], f32, tag="s", bufs=3)
                        for h in range(HEADS_PER_CORE):
                            nc.tensor.matmul(yp[:],
                                             attn_t[:, h, ts(tq, 128)],
                                             wo[h][:, ts(nn, 512)],
                                             start=(h == 0), stop=(h == 3))
                        ysb = y_pool.tile([128, 512], f32, tag="y")
                        nc.vector.tensor_copy(ysb[:], yp[:])
                        nc.sync.dma_start(
                            y_d[row0:row0 + 128, ts(nn, 512)], ysb[:])

            # ---- emission order: interleave so attention/Wo of chunk c
            # overlap projections of chunk c+2 ----
            for c in range(NCQ):
                proj_chunk(c)
                at = attn_chunk(c)
                wo_chunk(c, at)

    nc.finalize()
    return nc


def _prep_inputs(x, Wq, Wk, Wv, Wo, t_len=T):
    """Host-side shard + layout prep -> per-core input maps."""
    import ml_dtypes
    bf16 = ml_dtypes.bfloat16

    x = np.asarray(x, np.float32)
    Wq = np.asarray(Wq, np.float32)
    Wk = np.asarray(Wk, np.float32)
    Wv = np.asarray(Wv, np.float32)
    Wo = np.asarray(Wo, np.float32)

    # RoPE de-interleave permutation within one head: [evens | odds]
    perm = np.concatenate([np.arange(0, HD, 2), np.arange(1, HD, 2)])

    # rope tables (match reference: freqs = t * base**(-2j/HD))
    inv = 1.0 / (ROPE_BASE ** (np.arange(0, HD, 2, dtype=np.float32) / HD))
    tpos = np.arange(t_len, dtype=np.float32)
    f = inv[:, None] * tpos[None, :]                       # [64, T]
    cos_dup = np.concatenate([np.cos(f), np.cos(f)], 0)    # [128, T]
    ssig = np.concatenate([-np.sin(f), np.sin(f)], 0)      # [128, T]
    cos_dup = cos_dup.astype(bf16)
    ssig = ssig.astype(bf16)

    # strict-lower-triangular causal mask template for the diagonal
    # [tk-tile, tq-tile] block (tk > tq within the 128x128 block)
    r = np.arange(128)[:, None]
    col = np.arange(128)[None, :]
    mask_t = np.where(r > col, MASK_VAL, 0.0).astype(bf16)

    in_maps = []
    for b in range(B):
        xT_b = np.ascontiguousarray(x[b, :t_len].T).astype(bf16)  # [D, T]
        for g in range(KV):
            wq_g = Wq[:, g * DQ:(g + 1) * DQ].reshape(D, HEADS_PER_CORE, HD)
            wq_g = np.ascontiguousarray(
                wq_g[:, :, perm].reshape(D, DQ)).astype(bf16)
            wk_g = np.ascontiguousarray(
                Wk[:, g * HD:(g + 1) * HD][:, perm]).astype(bf16)
            wv_g = np.ascontiguousarray(
                Wv[:, g * HD:(g + 1) * HD]).astype(bf16)
            wo_g = np.ascontiguousarray(
                Wo[g * DQ:(g + 1) * DQ, :]).astype(bf16)
            in_maps.append({
                "xT": xT_b, "wq": wq_g, "wk": wk_g, "wv": wv_g,
                "wo": wo_g, "cosd": cos_dup, "ssig": ssig, "mask": mask_t,
            })
    return in_maps


def run(inputs, trace=False, t_len=T):
    """Run the sharded kernel; returns (y_full, BassKernelResults)."""
    from concourse.bass_utils import run_bass_kernel_spmd

    key = ("nc", t_len)
    if key not in _CACHE:
        _CACHE[key] = _build_nc(t_len)
    nc = _CACHE[key]

    in_maps = _prep_inputs(inputs["x"], inputs["Wq"], inputs["Wk"],
                           inputs["Wv"], inputs["Wo"], t_len)
    res = run_bass_kernel_spmd(nc, in_maps, list(range(N_CORES)), trace=trace)

    y = np.empty((B, t_len, D), np.float32)
    for b in range(B):
        acc = np.zeros((t_len, D), np.float32)
        for g in range(KV):
            acc += np.asarray(res.results[b * KV + g]["y"], np.float32)
        y[b] = acc
    return y, res


def kernel(**inputs) -> np.ndarray:
    y, _ = run(inputs, trace=False)
    return y
